# revision 2
# baseline (speedup 1.0000x reference)
"""Differentiable-JPEG TRN2 kernel v2 (8-core data-parallel, full I/O).

Key ideas vs v1:
- Linearity: out = clip(x + MINV@IDCT2(uu)), uu = 0.5*q*tanh(15*C/q).
  Only the nonlinear correction rides the IDCT path; the identity term is
  added back by a PE identity-matmul accumulate. No cq add, no on-chip clip
  (host clips the f16 output).
- fp16 input upload + fp16/f32r matmuls everywhere (1 cyc/row vs 4 for f32).
- d15 (C * 15/q) folded into MM2: 24 matmuls with per-(c,u) v-scaled
  block-diag DCT stationaries -> tanh reads PSUM directly, no DVE multiply.
- Engine placement: ACT: tanh + out-copy; DVE: T-copy + R-copy; Pool: uu.
"""
import numpy as np

B, C, H, W = 32, 3, 512, 512
NCORES = 8
BPC = B // NCORES           # images per core
G, CCH, XX = 4, 3, 8        # 8-row groups per tile, channels, rows per block
P96 = G * CCH * XX          # 96 packed partitions
NT = H // 32                # 16 h-tiles per image
FREE = NT * W               # 8192 free elements per image buffer

QUALITY = 50.0
_LUM = np.array([[16,11,10,16,24,40,51,61],[12,12,14,19,26,58,60,55],[14,13,16,24,40,57,69,56],[14,17,22,29,51,87,80,62],[18,22,37,56,68,109,103,77],[24,35,55,64,81,104,113,92],[49,64,78,87,103,121,120,101],[72,92,95,98,112,100,103,99]], dtype=np.float32)
_CHR = np.array([[17,18,24,47,99,99,99,99],[18,21,26,66,99,99,99,99],[24,26,56,99,99,99,99,99],[47,66,99,99,99,99,99,99],[99,99,99,99,99,99,99,99],[99,99,99,99,99,99,99,99],[99,99,99,99,99,99,99,99],[99,99,99,99,99,99,99,99]], dtype=np.float32)


def _scaled_qtable(base, qf):
    qf = max(1.0, min(100.0, qf))
    s = 5000.0 / qf if qf < 50 else 200.0 - 2.0 * qf
    return np.maximum(np.floor((base * s + 50.0) / 100.0), 1.0)


def _np_consts():
    qtab = np.stack([_scaled_qtable(_LUM, QUALITY), _scaled_qtable(_CHR, QUALITY),
                     _scaled_qtable(_CHR, QUALITY)]).astype(np.float32)  # [c,u,v]
    u8 = np.arange(8)[:, None]
    x8 = np.arange(8)[None, :]
    cu = np.where(u8 == 0, 1.0 / np.sqrt(2.0), 1.0)
    D = (0.5 * cu * np.cos((2 * x8 + 1) * u8 * np.pi / 16.0)).astype(np.float32)
    MFWD = np.array([[0.299, 0.587, 0.114], [-0.168736, -0.331264, 0.5],
                     [0.5, -0.418688, -0.081312]], np.float32)
    MINV = np.array([[1.0, 0.0, 1.402], [1.0, -0.344136, -0.714136],
                     [1.0, 1.772, 0.0]], np.float32)

    # A1 [97, 96]: color fwd + H-DCT (+ -0.5 DC bias on Y via ones row).
    # col order n = (c2, g, u)  (baseline order: n = c2*32 + g*8 + u)
    A1 = np.zeros((97, 96), np.float32)
    AI2 = np.zeros((96, 96), np.float32)
    for g in range(G):
        for c in range(CCH):
            for c2 in range(CCH):
                p0 = c * 32 + g * 8
                n0 = c2 * 32 + g * 8
                A1[p0:p0 + 8, n0:n0 + 8] = MFWD[c2, c] * D.T
                AI2[n0:n0 + 8, p0:p0 + 8] = MINV[c, c2] * D
        A1[96, g * 8] = -np.sqrt(2.0)   # (c2=Y, u=0): forward -0.5 pixel bias

    # BDt [128,128]: block-diag D^T  (maps w=(wb,y) -> (wb,v))
    BDt = np.zeros((128, 128), np.float32)
    for a in range(16):
        BDt[8 * a:8 * a + 8, 8 * a:8 * a + 8] = D.T
    BD = np.ascontiguousarray(BDt.T)

    # 24 per-(c,u) v-scaled copies of BDt: column (wb,v) scaled by 15/q[c,u,v]
    # packed as one [128, 24*128] tensor, slice cu gives the stationary.
    v = np.arange(128) % 8
    BDQ = np.zeros((128, 24 * 128), np.float32)
    for c in range(CCH):
        for u in range(XX):
            cu = c * 8 + u
            BDQ[:, cu * 128:(cu + 1) * 128] = BDt * (15.0 / qtab[c, u, v])[None, :]

    # HQ [128, 384]: 0.5*q in the D15/tt layout [(wb,v), (c,u,j,g)]
    HQ = np.zeros((128, 384), np.float32)
    for c in range(CCH):
        for u in range(XX):
            for j in range(4):
                for g in range(G):
                    col = (c * 8 + u) * 16 + j * 4 + g
                    HQ[:, col] = 0.5 * qtab[c, u, v]

    I96 = np.eye(96, dtype=np.float32)
    ONES = np.ones((1, FREE), dtype=np.float32)
    return {"a1": A1, "bdq": BDQ, "bd": BD, "hq": HQ, "ai2": AI2, "i96": I96,
            "ones": ONES}


_CACHE = {}


def _build(tcopy="vector", rcopy="vector", outcopy="scalar", uu_eng="gpsimd",
           pe_add=True, work_bufs=3, split_loads=False, n_images=BPC, **kw):
    import concourse.bacc as bacc
    import concourse.mybir as mybir
    import concourse.tile as tile

    F32 = mybir.dt.float32
    F32R = mybir.dt.float32r
    F16 = mybir.dt.float16
    AOT = mybir.AluOpType
    ACTF = mybir.ActivationFunctionType
    nc = bacc.Bacc("TRN2", target_bir_lowering=False, debug=False)

    x = nc.dram_tensor("x", [BPC, C, H, W], F16, kind="ExternalInput")
    out = nc.dram_tensor("out", [BPC, C, H, W], F16, kind="ExternalOutput")
    cdtype = {"a1": F16, "bdq": F16, "bd": F16, "hq": F16, "ai2": F32R,
              "i96": F16, "ones": F16}
    cd = {k: nc.dram_tensor(k, list(vv.shape), cdtype[k], kind="ExternalInput")
          for k, vv in _np_consts().items()}

    xin_src = x.ap().rearrange("b c (t g xx) w -> b c (g xx) t w", t=NT, g=G, xx=XX)
    out_dst = out.ap().rearrange("b c (t g xx) w -> b c (g xx) t w", t=NT, g=G, xx=XX)

    with tile.TileContext(nc) as tc:
        csb = {k: nc.alloc_sbuf_tensor(f"c_{k}", list(v.shape), cdtype[k])
               for k, v in _np_consts().items() if k != "ones"}
        xin = [nc.alloc_sbuf_tensor(f"xin{i}", [97, FREE], F16) for i in range(3)]
        rout = [nc.alloc_sbuf_tensor(f"rout{i}", [P96, FREE], F16) for i in range(2)]
        zbias = nc.alloc_sbuf_tensor("zbias", [128, 1], F32)

        nc.sync.dma_start(out=xin[0].ap()[96:97, :], in_=cd["ones"].ap())
        nc.sync.dma_start(out=csb["a1"].ap(), in_=cd["a1"].ap())
        _preamble_rest = [k for k in csb if k != "a1"]
        nc.vector.memset(zbias.ap(), 0.0)

        a1, bdq, bd = csb["a1"].ap(), csb["bdq"].ap(), csb["bd"].ap()
        hq, ai2, i96 = csb["hq"].ap(), csb["ai2"].ap(), csb["i96"].ap()
        zb = zbias.ap()

        with (
            tc.tile_pool(name="psT", bufs=2, space="PSUM") as psT,
            tc.tile_pool(name="psD", bufs=2, space="PSUM") as psD,
            tc.tile_pool(name="psR", bufs=2, space="PSUM") as psR,
            tc.tile_pool(name="psY", bufs=2, space="PSUM") as psY,
            tc.tile_pool(name="work", bufs=work_bufs) as work,
        ):
            def load_image(b, tchunk=NT, t_from=0, t_to=NT):
                xv = xin[b % 3].ap()
                for t0 in range(t_from, t_to, tchunk):
                    tn = min(tchunk, t_to - t0)
                    for c in range(CCH):
                        nc.sync.dma_start(
                            out=xv[c * 32:(c + 1) * 32,
                                   t0 * W:(t0 + tn) * W].rearrange(
                                "p (t w) -> p t w", t=tn),
                            in_=xin_src[b, c, :, t0:t0 + tn])

            def store_half(b, h0):
                ov = rout[b % 2].ap()
                t0 = h0 * (NT // 2)
                for c in range(CCH):
                    nc.sync.dma_start(
                        out=out_dst[b, c, :, t0:t0 + NT // 2],
                        in_=ov[c * 32:(c + 1) * 32,
                               t0 * W:(t0 + NT // 2) * W].rearrange(
                            "p (t w) -> p t w", t=NT // 2))

            items = [(b, t) for b in range(n_images) for t in range(NT)]
            NI = len(items)
            st = {}

            if split_loads:
                load_image(0, tchunk=2, t_to=4)
                load_image(0, tchunk=4, t_from=4)
            else:
                load_image(0)
            for k in _preamble_rest:
                nc.sync.dma_start(out=csb[k].ap(), in_=cd[k].ap())
            for i in range(1, len(xin)):
                nc.sync.dma_start(out=xin[i].ap()[96:97, :],
                                  in_=cd["ones"].ap())
            if BPC > 1:
                load_image(1)

            for s in range(NI + 9):
                # ---- PE ----
                if s < NI:
                    b, t = items[s]
                    xv = xin[b % 3].ap()
                    base = t * W
                    T_ps = psT.tile([128, 384], F32)
                    for j in range(4):
                        nc.tensor.matmul(
                            T_ps[:, 96 * j:96 * j + 96],
                            xv[0:97, base + 128 * j:base + 128 * j + 128],
                            a1, start=True, stop=True)
                    st[s] = {"T_ps": T_ps, "b": b, "t": t}
                if 0 <= s - 2 < NI:
                    e = st[s - 2]
                    D_ps = psD.tile([128, 384], F32)
                    for c in range(CCH):
                        for u in range(XX):
                            cu = c * 8 + u
                            nc.tensor.matmul(
                                D_ps[:, cu * 16:(cu + 1) * 16],
                                bdq[:, cu * 128:(cu + 1) * 128],
                                _t_cols(e["t_sb"], c, u),
                                start=True, stop=True)
                    e["D_ps"] = D_ps
                if 0 <= s - 5 < NI:
                    e = st[s - 5]
                    R_ps = psR.tile([P96, W], F32)
                    for j in range(4):
                        nc.tensor.matmul(
                            R_ps[:, 128 * j:128 * j + 128],
                            e["uu"][:, 96 * j:96 * j + 96],
                            bd, start=True, stop=True)
                    e["R_ps"] = R_ps
                if 0 <= s - 7 < NI:
                    e = st[s - 7]
                    b, t = e["b"], e["t"]
                    xv = xin[b % 3].ap()
                    Y_ps = psY.tile([P96, W], F32)
                    if pe_add:
                        nc.tensor.matmul(Y_ps[:, :], ai2, e["rv"][0:P96, :],
                                         start=True, stop=False)
                        nc.tensor.matmul(Y_ps[:, :], i96,
                                         xv[0:P96, t * W:(t + 1) * W],
                                         start=False, stop=True)
                    else:
                        nc.tensor.matmul(Y_ps[:, :], ai2, e["rv"][0:P96, :],
                                         start=True, stop=True)
                    e["Y_ps"] = Y_ps

                def eng_copy(eng, dst, src_ap):
                    if eng == "vector":
                        nc.vector.tensor_copy(dst, src_ap)
                    else:
                        nc.scalar.copy(dst, src_ap)

                if 0 <= s - 1 < NI:
                    e = st[s - 1]
                    t_sb = work.tile([128, 384], F16, tag="t_sb")
                    eng_copy(tcopy, t_sb, e["T_ps"][:, :])
                    e["t_sb"] = t_sb
                if 0 <= s - 6 < NI:
                    e = st[s - 6]
                    rv = work.tile([P96, W], F32R, tag="rv")
                    eng_copy(rcopy, rv, e["R_ps"][:, :])
                    e["rv"] = rv

                if 0 <= s - 3 < NI:
                    e = st[s - 3]
                    tt = work.tile([128, 384], F16, tag="tt")
                    nc.scalar.activation(tt, e["D_ps"][:, :], ACTF.Tanh,
                                         bias=zb, scale=1.0)
                    e["tt"] = tt
                if 0 <= s - 8 < NI:
                    e = st[s - 8]
                    ov = rout[e["b"] % 2].ap()
                    dst = ov[:, e["t"] * W:(e["t"] + 1) * W]
                    if pe_add:
                        eng_copy(outcopy, dst, e["Y_ps"][:, :])
                    else:
                        xv2 = xin[e["b"] % 3].ap()
                        nc.vector.tensor_tensor(
                            dst, e["Y_ps"][:, :],
                            xv2[0:P96, e["t"] * W:(e["t"] + 1) * W], AOT.add)
                    if e["t"] == NT // 2 - 1:
                        store_half(e["b"], 0)
                    elif e["t"] == NT - 1:
                        store_half(e["b"], 1)
                    del st[s - 8]

                if 0 <= s - 4 < NI:
                    e = st[s - 4]
                    uu = work.tile([128, 384], F16, tag="uu")
                    eng = nc.gpsimd if uu_eng == "gpsimd" else nc.vector
                    ttv = e["tt"].rearrange("p (c u j g) -> p c u j g",
                                            c=3, u=8, j=4)
                    hqv = hq.rearrange("p (c u j g) -> p c u j g", c=3, u=8, j=4)
                    uuv = uu.rearrange("p (j c g u) -> p c u j g", j=4, c=3, g=4)
                    for c in range(CCH):
                        eng.tensor_tensor(uuv[:, c], ttv[:, c], hqv[:, c],
                                          AOT.mult)
                    e["uu"] = uu

                # prefetch image b+2 only once image b's LAST identity-add
                # (the final xin[b] reader) has been emitted -- the tile
                # framework orders the buffer overwrite after already-emitted
                # reads only.
                if 0 <= s - 7 < NI:
                    eb, et = items[s - 7]
                    if et == NT - 1 and eb + 2 < n_images:
                        load_image(eb + 2)
    nc.compile()
    return nc


def _t_cols(t_sb, c, u):
    """Moving operand for MM2_cu: t columns {(j, c, g, u) : j, g} as
    [128, (j:4 stride 96), (g:4 stride 8)] at offset c*32+u."""
    return t_sb.rearrange("p (j cc g u) -> p j cc g u", j=4, cc=3, g=4)[
        :, :, c, :, u]


def _get_nc(**kw):
    key = tuple(sorted(kw.items()))
    if key not in _CACHE:
        _CACHE[key] = _build(**kw)
    return _CACHE[key]


def kernel(x, trace=False, **kw):
    from concourse import bass_utils
    nc = _get_nc(**kw)
    consts = _np_consts()
    up = {}
    for k, v in consts.items():
        if k in ("ai2",):
            up[k] = v  # f32r = f32 bits
        else:
            up[k] = v.astype(np.float16)
    x = np.asarray(x)
    xh = x.astype(np.float16)
    in_maps = []
    for i in range(NCORES):
        m = {"x": xh[i * BPC:(i + 1) * BPC]}
        m.update(up)
        in_maps.append(m)
    res = bass_utils.run_bass_kernel_spmd(
        nc, in_maps, core_ids=list(range(NCORES)), trace=trace)
    _CACHE["last"] = res
    outs = np.concatenate([r["out"] for r in res.results], axis=0)
    return np.clip(outs.astype(np.float32), 0.0, 1.0)


def last_exec_time_ns():
    res = _CACHE.get("last")
    return None if res is None else res.exec_time_ns


# revision 3
# speedup vs baseline: 1.0058x; 1.0058x over previous
"""Differentiable-JPEG TRN2 kernel v2 (8-core data-parallel, full I/O).

Key ideas vs v1:
- Linearity: out = clip(x + MINV@IDCT2(uu)), uu = 0.5*q*tanh(15*C/q).
  Only the nonlinear correction rides the IDCT path; the identity term is
  added back by a PE identity-matmul accumulate. No cq add, no on-chip clip
  (host clips the f16 output).
- fp16 input upload + fp16/f32r matmuls everywhere (1 cyc/row vs 4 for f32).
- d15 (C * 15/q) folded into MM2: 24 matmuls with per-(c,u) v-scaled
  block-diag DCT stationaries -> tanh reads PSUM directly, no DVE multiply.
- Engine placement: ACT: tanh + out-copy; DVE: T-copy + R-copy;
  Pool: uu multiply (3 per-channel strided ops).
"""
import numpy as np

B, C, H, W = 32, 3, 512, 512
NCORES = 8
BPC = B // NCORES           # images per core
G, CCH, XX = 4, 3, 8        # 8-row groups per tile, channels, rows per block
P96 = G * CCH * XX          # 96 packed partitions
NT = H // 32                # 16 h-tiles per image
FREE = NT * W               # 8192 free elements per image buffer

QUALITY = 50.0
_LUM = np.array([[16,11,10,16,24,40,51,61],[12,12,14,19,26,58,60,55],[14,13,16,24,40,57,69,56],[14,17,22,29,51,87,80,62],[18,22,37,56,68,109,103,77],[24,35,55,64,81,104,113,92],[49,64,78,87,103,121,120,101],[72,92,95,98,112,100,103,99]], dtype=np.float32)
_CHR = np.array([[17,18,24,47,99,99,99,99],[18,21,26,66,99,99,99,99],[24,26,56,99,99,99,99,99],[47,66,99,99,99,99,99,99],[99,99,99,99,99,99,99,99],[99,99,99,99,99,99,99,99],[99,99,99,99,99,99,99,99],[99,99,99,99,99,99,99,99]], dtype=np.float32)


def _scaled_qtable(base, qf):
    qf = max(1.0, min(100.0, qf))
    s = 5000.0 / qf if qf < 50 else 200.0 - 2.0 * qf
    return np.maximum(np.floor((base * s + 50.0) / 100.0), 1.0)


def _np_consts():
    qtab = np.stack([_scaled_qtable(_LUM, QUALITY), _scaled_qtable(_CHR, QUALITY),
                     _scaled_qtable(_CHR, QUALITY)]).astype(np.float32)  # [c,u,v]
    u8 = np.arange(8)[:, None]
    x8 = np.arange(8)[None, :]
    cu = np.where(u8 == 0, 1.0 / np.sqrt(2.0), 1.0)
    D = (0.5 * cu * np.cos((2 * x8 + 1) * u8 * np.pi / 16.0)).astype(np.float32)
    MFWD = np.array([[0.299, 0.587, 0.114], [-0.168736, -0.331264, 0.5],
                     [0.5, -0.418688, -0.081312]], np.float32)
    MINV = np.array([[1.0, 0.0, 1.402], [1.0, -0.344136, -0.714136],
                     [1.0, 1.772, 0.0]], np.float32)

    # A1 [97, 96]: color fwd + H-DCT (+ -0.5 DC bias on Y via ones row).
    # col order n = (c2, g, u)  (baseline order: n = c2*32 + g*8 + u)
    A1 = np.zeros((97, 96), np.float32)
    AI2 = np.zeros((96, 96), np.float32)
    for g in range(G):
        for c in range(CCH):
            for c2 in range(CCH):
                p0 = c * 32 + g * 8
                n0 = c2 * 32 + g * 8
                A1[p0:p0 + 8, n0:n0 + 8] = MFWD[c2, c] * D.T
                AI2[n0:n0 + 8, p0:p0 + 8] = MINV[c, c2] * D
        A1[96, g * 8] = -np.sqrt(2.0)   # (c2=Y, u=0): forward -0.5 pixel bias

    # BDt [128,128]: block-diag D^T  (maps w=(wb,y) -> (wb,v))
    BDt = np.zeros((128, 128), np.float32)
    for a in range(16):
        BDt[8 * a:8 * a + 8, 8 * a:8 * a + 8] = D.T
    BD = np.ascontiguousarray(BDt.T)

    # 24 per-(c,u) v-scaled copies of BDt: column (wb,v) scaled by 15/q[c,u,v]
    # packed as one [128, 24*128] tensor, slice cu gives the stationary.
    v = np.arange(128) % 8
    BDQ = np.zeros((128, 24 * 128), np.float32)
    for c in range(CCH):
        for u in range(XX):
            cu = c * 8 + u
            BDQ[:, cu * 128:(cu + 1) * 128] = BDt * (15.0 / qtab[c, u, v])[None, :]

    # HQ [128, 384]: 0.5*q in the D15/tt layout [(wb,v), (c,u,j,g)]
    HQ = np.zeros((128, 384), np.float32)
    for c in range(CCH):
        for u in range(XX):
            for j in range(4):
                for g in range(G):
                    col = (c * 8 + u) * 16 + j * 4 + g
                    HQ[:, col] = 0.5 * qtab[c, u, v]

    I96 = np.eye(96, dtype=np.float32)
    ONES = np.ones((1, FREE), dtype=np.float32)
    return {"a1": A1, "bdq": BDQ, "bd": BD, "hq": HQ, "ai2": AI2, "i96": I96,
            "ones": ONES}


_CACHE = {}


def _build(tcopy="vector", rcopy="vector", outcopy="scalar", uu_eng="gpsimd",
           pe_add=True, work_bufs=3, split_loads=True, n_images=BPC, tight=False, **kw):
    import concourse.bacc as bacc
    import concourse.mybir as mybir
    import concourse.tile as tile

    F32 = mybir.dt.float32
    F32R = mybir.dt.float32r
    F16 = mybir.dt.float16
    AOT = mybir.AluOpType
    ACTF = mybir.ActivationFunctionType
    nc = bacc.Bacc("TRN2", target_bir_lowering=False, debug=False)

    x = nc.dram_tensor("x", [BPC, C, H, W], F16, kind="ExternalInput")
    out = nc.dram_tensor("out", [BPC, C, H, W], F16, kind="ExternalOutput")
    cdtype = {"a1": F16, "bdq": F16, "bd": F16, "hq": F16, "ai2": F32R,
              "i96": F16, "ones": F16}
    cd = {k: nc.dram_tensor(k, list(vv.shape), cdtype[k], kind="ExternalInput")
          for k, vv in _np_consts().items()}

    xin_src = x.ap().rearrange("b c (t g xx) w -> b c (g xx) t w", t=NT, g=G, xx=XX)
    out_dst = out.ap().rearrange("b c (t g xx) w -> b c (g xx) t w", t=NT, g=G, xx=XX)

    with tile.TileContext(nc) as tc:
        csb = {k: nc.alloc_sbuf_tensor(f"c_{k}", list(v.shape), cdtype[k])
               for k, v in _np_consts().items() if k != "ones"}
        xin = [nc.alloc_sbuf_tensor(f"xin{i}", [97, FREE], F16) for i in range(3)]
        rout = [nc.alloc_sbuf_tensor(f"rout{i}", [P96, FREE], F16) for i in range(2)]
        zbias = nc.alloc_sbuf_tensor("zbias", [128, 1], F32)

        nc.sync.dma_start(out=xin[0].ap()[96:97, :], in_=cd["ones"].ap())
        nc.sync.dma_start(out=csb["a1"].ap(), in_=cd["a1"].ap())
        _preamble_rest = [k for k in csb if k != "a1"]
        nc.vector.memset(zbias.ap(), 0.0)

        a1, bdq, bd = csb["a1"].ap(), csb["bdq"].ap(), csb["bd"].ap()
        hq, ai2, i96 = csb["hq"].ap(), csb["ai2"].ap(), csb["i96"].ap()
        zb = zbias.ap()

        with (
            tc.tile_pool(name="psT", bufs=2, space="PSUM") as psT,
            tc.tile_pool(name="psD", bufs=2, space="PSUM") as psD,
            tc.tile_pool(name="psR", bufs=2, space="PSUM") as psR,
            tc.tile_pool(name="psY", bufs=2, space="PSUM") as psY,
            tc.tile_pool(name="work", bufs=work_bufs) as work,
        ):
            def load_image(b, tchunk=NT, t_from=0, t_to=NT):
                xv = xin[b % 3].ap()
                for t0 in range(t_from, t_to, tchunk):
                    tn = min(tchunk, t_to - t0)
                    for c in range(CCH):
                        nc.sync.dma_start(
                            out=xv[c * 32:(c + 1) * 32,
                                   t0 * W:(t0 + tn) * W].rearrange(
                                "p (t w) -> p t w", t=tn),
                            in_=xin_src[b, c, :, t0:t0 + tn])

            def store_chunk(b, t0, tn):
                ov = rout[b % 2].ap()
                for c in range(CCH):
                    nc.sync.dma_start(
                        out=out_dst[b, c, :, t0:t0 + tn],
                        in_=ov[c * 32:(c + 1) * 32,
                               t0 * W:(t0 + tn) * W].rearrange(
                            "p (t w) -> p t w", t=tn))

            if tight is True:
                OFF = {"tcopy": 0, "mm2": 1, "tanh": 2, "uu": 3,
                       "mm3": 4, "rcopy": 4, "mm4": 5, "outcopy": 5}
            elif isinstance(tight, dict):
                OFF = tight
            else:
                OFF = {"tcopy": 1, "mm2": 2, "tanh": 3, "uu": 4,
                       "mm3": 5, "rcopy": 6, "mm4": 7, "outcopy": 8}
            items = [(b, t) for b in range(n_images) for t in range(NT)]
            NI = len(items)
            st = {}

            if split_loads:
                load_image(0, t_to=4)
                nc.sync.dma_start(out=csb["bdq"].ap(), in_=cd["bdq"].ap())
                load_image(0, t_from=4)
                rest = [k for k in _preamble_rest if k != "bdq"]
            else:
                load_image(0)
                rest = _preamble_rest
            for k in rest:
                nc.sync.dma_start(out=csb[k].ap(), in_=cd[k].ap())
            for i in range(1, len(xin)):
                nc.sync.dma_start(out=xin[i].ap()[96:97, :],
                                  in_=cd["ones"].ap())
            if BPC > 1:
                load_image(1)

            for s in range(NI + OFF["outcopy"] + 1):
                # ---- PE ----
                if s < NI:
                    b, t = items[s]
                    xv = xin[b % 3].ap()
                    base = t * W
                    T_ps = psT.tile([128, 384], F32)
                    for j in range(4):
                        nc.tensor.matmul(
                            T_ps[:, 96 * j:96 * j + 96],
                            xv[0:97, base + 128 * j:base + 128 * j + 128],
                            a1, start=True, stop=True)
                    st[s] = {"T_ps": T_ps, "b": b, "t": t}
                if 0 <= s - OFF["mm2"] < NI:
                    e = st[s - OFF["mm2"]]
                    D_ps = psD.tile([128, 384], F32)
                    for c in range(CCH):
                        for u in range(XX):
                            cu = c * 8 + u
                            nc.tensor.matmul(
                                D_ps[:, cu * 16:(cu + 1) * 16],
                                bdq[:, cu * 128:(cu + 1) * 128],
                                _t_cols(e["t_sb"], c, u),
                                start=True, stop=True)
                    e["D_ps"] = D_ps
                if 0 <= s - OFF["mm3"] < NI:
                    e = st[s - OFF["mm3"]]
                    R_ps = psR.tile([P96, W], F32)
                    for j in range(4):
                        nc.tensor.matmul(
                            R_ps[:, 128 * j:128 * j + 128],
                            e["uu"][:, 96 * j:96 * j + 96],
                            bd, start=True, stop=True)
                    e["R_ps"] = R_ps
                if 0 <= s - OFF["mm4"] < NI:
                    e = st[s - OFF["mm4"]]
                    b, t = e["b"], e["t"]
                    xv = xin[b % 3].ap()
                    Y_ps = psY.tile([P96, W], F32)
                    if pe_add:
                        nc.tensor.matmul(Y_ps[:, :], ai2, e["rv"][0:P96, :],
                                         start=True, stop=False)
                        nc.tensor.matmul(Y_ps[:, :], i96,
                                         xv[0:P96, t * W:(t + 1) * W],
                                         start=False, stop=True)
                    else:
                        nc.tensor.matmul(Y_ps[:, :], ai2, e["rv"][0:P96, :],
                                         start=True, stop=True)
                    e["Y_ps"] = Y_ps

                def eng_copy(eng, dst, src_ap):
                    if eng == "vector":
                        nc.vector.tensor_copy(dst, src_ap)
                    else:
                        nc.scalar.copy(dst, src_ap)

                if 0 <= s - OFF["tcopy"] < NI:
                    e = st[s - OFF["tcopy"]]
                    t_sb = work.tile([128, 384], F16, tag="t_sb")
                    eng_copy(tcopy, t_sb, e["T_ps"][:, :])
                    e["t_sb"] = t_sb
                if 0 <= s - OFF["rcopy"] < NI:
                    e = st[s - OFF["rcopy"]]
                    rv = work.tile([P96, W], F32R, tag="rv")
                    eng_copy(rcopy, rv, e["R_ps"][:, :])
                    e["rv"] = rv

                if 0 <= s - OFF["tanh"] < NI:
                    e = st[s - OFF["tanh"]]
                    tt = work.tile([128, 384], F16, tag="tt")
                    nc.scalar.activation(tt, e["D_ps"][:, :], ACTF.Tanh,
                                         bias=zb, scale=1.0)
                    e["tt"] = tt
                if 0 <= s - OFF["outcopy"] < NI:
                    e = st[s - OFF["outcopy"]]
                    ov = rout[e["b"] % 2].ap()
                    dst = ov[:, e["t"] * W:(e["t"] + 1) * W]
                    if pe_add:
                        eng_copy(outcopy, dst, e["Y_ps"][:, :])
                    else:
                        xv2 = xin[e["b"] % 3].ap()
                        nc.vector.tensor_tensor(
                            dst, e["Y_ps"][:, :],
                            xv2[0:P96, e["t"] * W:(e["t"] + 1) * W], AOT.add)
                    if e["t"] % 4 == 3:
                        store_chunk(e["b"], e["t"] - 3, 4)
                    del st[s - OFF["outcopy"]]

                if 0 <= s - OFF["uu"] < NI:
                    e = st[s - OFF["uu"]]
                    uu = work.tile([128, 384], F16, tag="uu")
                    eng = nc.gpsimd if uu_eng == "gpsimd" else nc.vector
                    ttv = e["tt"].rearrange("p (c u j g) -> p c u j g",
                                            c=3, u=8, j=4)
                    hqv = hq.rearrange("p (c u j g) -> p c u j g", c=3, u=8, j=4)
                    uuv = uu.rearrange("p (j c g u) -> p c u j g", j=4, c=3, g=4)
                    for c in range(CCH):
                        eng.tensor_tensor(uuv[:, c], ttv[:, c], hqv[:, c],
                                          AOT.mult)
                    e["uu"] = uu

                # prefetch image b+2 only once image b's LAST identity-add
                # (the final xin[b] reader) has been emitted -- the tile
                # framework orders the buffer overwrite after already-emitted
                # reads only.
                if 0 <= s - OFF["mm4"] < NI:
                    eb, et = items[s - OFF["mm4"]]
                    if et == NT - 1 and eb + 2 < n_images:
                        load_image(eb + 2)
    nc.compile()
    return nc


def _t_cols(t_sb, c, u):
    """Moving operand for MM2_cu: t columns {(j, c, g, u) : j, g} as
    [128, (j:4 stride 96), (g:4 stride 8)] at offset c*32+u."""
    return t_sb.rearrange("p (j cc g u) -> p j cc g u", j=4, cc=3, g=4)[
        :, :, c, :, u]


def _get_nc(**kw):
    key = tuple(sorted(kw.items()))
    if key not in _CACHE:
        _CACHE[key] = _build(**kw)
    return _CACHE[key]


def kernel(x, trace=False, **kw):
    from concourse import bass_utils
    nc = _get_nc(**kw)
    consts = _np_consts()
    up = {}
    for k, v in consts.items():
        if k in ("ai2",):
            up[k] = v  # f32r = f32 bits
        else:
            up[k] = v.astype(np.float16)
    x = np.asarray(x)
    xh = x.astype(np.float16)
    in_maps = []
    for i in range(NCORES):
        m = {"x": xh[i * BPC:(i + 1) * BPC]}
        m.update(up)
        in_maps.append(m)
    res = bass_utils.run_bass_kernel_spmd(
        nc, in_maps, core_ids=list(range(NCORES)), trace=trace)
    _CACHE["last"] = res
    outs = np.concatenate([r["out"] for r in res.results], axis=0)
    return np.clip(outs.astype(np.float32), 0.0, 1.0)


def last_exec_time_ns():
    res = _CACHE.get("last")
    return None if res is None else res.exec_time_ns


# revision 5
# speedup vs baseline: 1.0858x; 1.0795x over previous
"""Differentiable-JPEG TRN2 kernel v2 (8-core data-parallel, full I/O).

Key ideas vs v1:
- Linearity: out = clip(x + MINV@IDCT2(uu)), uu = 0.5*q*tanh(15*C/q).
  Only the nonlinear correction rides the IDCT path; the identity term is
  added back by a PE identity-matmul accumulate. No cq add, no on-chip clip
  (host clips the f16 output).
- fp16 input upload + fp16/f32r matmuls everywhere (1 cyc/row vs 4 for f32).
- d15 (C * 15/q) folded into MM2: 24 matmuls with per-(c,u) v-scaled
  block-diag DCT stationaries -> tanh reads PSUM directly, no DVE multiply.
- Engine placement: ACT: tanh + out-copy; DVE: T-copy + R-copy;
  Pool: uu multiply (3 per-channel strided ops).
"""
import numpy as np

B, C, H, W = 32, 3, 512, 512
NCORES = 8
BPC = B // NCORES           # images per core
G, CCH, XX = 4, 3, 8        # 8-row groups per tile, channels, rows per block
P96 = G * CCH * XX          # 96 packed partitions
NT = H // 32                # 16 h-tiles per image
FREE = NT * W               # 8192 free elements per image buffer

QUALITY = 50.0
_LUM = np.array([[16,11,10,16,24,40,51,61],[12,12,14,19,26,58,60,55],[14,13,16,24,40,57,69,56],[14,17,22,29,51,87,80,62],[18,22,37,56,68,109,103,77],[24,35,55,64,81,104,113,92],[49,64,78,87,103,121,120,101],[72,92,95,98,112,100,103,99]], dtype=np.float32)
_CHR = np.array([[17,18,24,47,99,99,99,99],[18,21,26,66,99,99,99,99],[24,26,56,99,99,99,99,99],[47,66,99,99,99,99,99,99],[99,99,99,99,99,99,99,99],[99,99,99,99,99,99,99,99],[99,99,99,99,99,99,99,99],[99,99,99,99,99,99,99,99]], dtype=np.float32)


def _scaled_qtable(base, qf):
    qf = max(1.0, min(100.0, qf))
    s = 5000.0 / qf if qf < 50 else 200.0 - 2.0 * qf
    return np.maximum(np.floor((base * s + 50.0) / 100.0), 1.0)


def _np_consts():
    qtab = np.stack([_scaled_qtable(_LUM, QUALITY), _scaled_qtable(_CHR, QUALITY),
                     _scaled_qtable(_CHR, QUALITY)]).astype(np.float32)  # [c,u,v]
    u8 = np.arange(8)[:, None]
    x8 = np.arange(8)[None, :]
    cu = np.where(u8 == 0, 1.0 / np.sqrt(2.0), 1.0)
    D = (0.5 * cu * np.cos((2 * x8 + 1) * u8 * np.pi / 16.0)).astype(np.float32)
    MFWD = np.array([[0.299, 0.587, 0.114], [-0.168736, -0.331264, 0.5],
                     [0.5, -0.418688, -0.081312]], np.float32)
    MINV = np.array([[1.0, 0.0, 1.402], [1.0, -0.344136, -0.714136],
                     [1.0, 1.772, 0.0]], np.float32)

    # A1 [97, 96]: color fwd + H-DCT (+ -0.5 DC bias on Y via ones row).
    # col order n = (c2, g, u)  (baseline order: n = c2*32 + g*8 + u)
    A1 = np.zeros((97, 96), np.float32)
    AI2 = np.zeros((96, 96), np.float32)
    for g in range(G):
        for c in range(CCH):
            for c2 in range(CCH):
                p0 = c * 32 + g * 8
                n0 = c2 * 32 + g * 8
                A1[p0:p0 + 8, n0:n0 + 8] = MFWD[c2, c] * D.T
                AI2[n0:n0 + 8, p0:p0 + 8] = MINV[c, c2] * D
        A1[96, g * 8] = -np.sqrt(2.0)   # (c2=Y, u=0): forward -0.5 pixel bias

    # BDt [128,128]: block-diag D^T  (maps w=(wb,y) -> (wb,v))
    BDt = np.zeros((128, 128), np.float32)
    for a in range(16):
        BDt[8 * a:8 * a + 8, 8 * a:8 * a + 8] = D.T
    BD = np.ascontiguousarray(BDt.T)

    # 24 per-(c,u) v-scaled copies of BDt: column (wb,v) scaled by 15/q[c,u,v]
    # packed as one [128, 24*128] tensor, slice cu gives the stationary.
    v = np.arange(128) % 8
    BDQ = np.zeros((128, 24 * 128), np.float32)
    for c in range(CCH):
        for u in range(XX):
            cu = c * 8 + u
            BDQ[:, cu * 128:(cu + 1) * 128] = BDt * (15.0 / qtab[c, u, v])[None, :]

    # HQ [128, 384]: 0.5*q in the D15/tt layout [(wb,v), (c,u,j,g)]
    HQ = np.zeros((128, 384), np.float32)
    for c in range(CCH):
        for u in range(XX):
            for j in range(4):
                for g in range(G):
                    col = (c * 8 + u) * 16 + j * 4 + g
                    HQ[:, col] = 0.5 * qtab[c, u, v]

    I96 = np.eye(96, dtype=np.float32)
    ONES = np.ones((1, FREE), dtype=np.float32)
    return {"a1": A1, "bdq": BDQ, "bd": BD, "hq": HQ, "ai2": AI2, "i96": I96,
            "ones": ONES}


_CACHE = {}


def _build(tcopy="vector", rcopy="vector", outcopy="scalar", uu_eng="gpsimd",
           pe_add=True, work_bufs=3, split_loads=True, n_images=BPC, tight=False,
           fused_tr=True, **kw):
    import concourse.bacc as bacc
    import concourse.mybir as mybir
    import concourse.tile as tile

    F32 = mybir.dt.float32
    F32R = mybir.dt.float32r
    F16 = mybir.dt.float16
    AOT = mybir.AluOpType
    ACTF = mybir.ActivationFunctionType
    nc = bacc.Bacc("TRN2", target_bir_lowering=False, debug=False)

    x = nc.dram_tensor("x", [BPC, C, H, W], F16, kind="ExternalInput")
    out = nc.dram_tensor("out", [BPC, C, H, W], F16, kind="ExternalOutput")
    cdtype = {"a1": F16, "bdq": F16, "bd": F16, "hq": F16,
              "ai2": (F16 if fused_tr else F32R), "i96": F16, "ones": F16}
    cd = {k: nc.dram_tensor(k, list(vv.shape), cdtype[k], kind="ExternalInput")
          for k, vv in _np_consts().items()}

    xin_src = x.ap().rearrange("b c (t g xx) w -> b c (g xx) t w", t=NT, g=G, xx=XX)
    out_dst = out.ap().rearrange("b c (t g xx) w -> b c (g xx) t w", t=NT, g=G, xx=XX)

    with tile.TileContext(nc) as tc:
        csb = {k: nc.alloc_sbuf_tensor(f"c_{k}", list(v.shape), cdtype[k])
               for k, v in _np_consts().items() if k != "ones"}
        xin = [nc.alloc_sbuf_tensor(f"xin{i}", [97, FREE], F16) for i in range(3)]
        rout = [nc.alloc_sbuf_tensor(f"rout{i}", [P96, FREE], F16) for i in range(2)]
        zbias = nc.alloc_sbuf_tensor("zbias", [128, 1], F32)

        nc.sync.dma_start(out=xin[0].ap()[96:97, :], in_=cd["ones"].ap())
        nc.sync.dma_start(out=csb["a1"].ap(), in_=cd["a1"].ap())
        _preamble_rest = [k for k in csb if k != "a1"]
        nc.vector.memset(zbias.ap(), 0.0)

        a1, bdq, bd = csb["a1"].ap(), csb["bdq"].ap(), csb["bd"].ap()
        hq, ai2, i96 = csb["hq"].ap(), csb["ai2"].ap(), csb["i96"].ap()
        zb = zbias.ap()

        psb = kw.get("psb", (2, 2, 2, 2))
        if fused_tr:
            # two 2-bank parity tensors: cols [0:384] = T, [512:1024] = R;
            # parity = production step % 2. Flat [0:1024] fused read.
            psTR = [nc.alloc_psum_tensor(f"psTR{i}", [128, 1024], F32)
                    for i in range(2)]
            for i in range(2):
                nc.vector.memset(psTR[i].ap()[96:128, 384:896], 0.0)
        from contextlib import ExitStack
        _es = ExitStack()
        psT = psR = None
        if not fused_tr:
            psT = _es.enter_context(
                tc.tile_pool(name="psT", bufs=psb[0], space="PSUM"))
            psR = _es.enter_context(
                tc.tile_pool(name="psR", bufs=psb[2], space="PSUM"))
        with (
            _es,
            tc.tile_pool(name="psD", bufs=psb[1], space="PSUM") as psD,
            tc.tile_pool(name="psY", bufs=psb[3], space="PSUM") as psY,
            tc.tile_pool(name="work", bufs=work_bufs) as work,
        ):
            def load_image(b, tchunk=NT, t_from=0, t_to=NT):
                xv = xin[b % 3].ap()
                for t0 in range(t_from, t_to, tchunk):
                    tn = min(tchunk, t_to - t0)
                    for c in range(CCH):
                        nc.sync.dma_start(
                            out=xv[c * 32:(c + 1) * 32,
                                   t0 * W:(t0 + tn) * W].rearrange(
                                "p (t w) -> p t w", t=tn),
                            in_=xin_src[b, c, :, t0:t0 + tn])

            def store_chunk(b, t0, tn):
                ov = rout[b % 2].ap()
                for c in range(CCH):
                    nc.sync.dma_start(
                        out=out_dst[b, c, :, t0:t0 + tn],
                        in_=ov[c * 32:(c + 1) * 32,
                               t0 * W:(t0 + tn) * W].rearrange(
                            "p (t w) -> p t w", t=tn))

            if tight is True:
                OFF = {"tcopy": 0, "mm2": 1, "tanh": 2, "uu": 3,
                       "mm3": 4, "rcopy": 4, "mm4": 5, "outcopy": 5}
            elif isinstance(tight, dict):
                OFF = tight
            else:
                OFF = {"tcopy": 1, "mm2": 2, "tanh": 3, "uu": 4,
                       "mm3": 5, "rcopy": 6, "mm4": 7, "outcopy": 8}
            items = [(b, t) for b in range(n_images) for t in range(NT)]
            NI = len(items)
            st = {}

            if split_loads == "A" or split_loads is True:
                load_image(0, t_to=4)
                nc.sync.dma_start(out=csb["bdq"].ap(), in_=cd["bdq"].ap())
                load_image(0, t_from=4)
                rest = [k for k in _preamble_rest if k != "bdq"]
            elif split_loads == "B":
                load_image(0, t_to=4)
                load_image(0, t_from=4, t_to=8)
                nc.sync.dma_start(out=csb["bdq"].ap(), in_=cd["bdq"].ap())
                load_image(0, t_from=8)
                rest = [k for k in _preamble_rest if k != "bdq"]
            elif split_loads == "C":
                load_image(0, t_to=2)
                nc.sync.dma_start(out=csb["bdq"].ap(), in_=cd["bdq"].ap())
                load_image(0, t_from=2, t_to=8)
                load_image(0, t_from=8)
                rest = [k for k in _preamble_rest if k != "bdq"]
            else:
                load_image(0)
                rest = _preamble_rest
            for k in rest:
                nc.sync.dma_start(out=csb[k].ap(), in_=cd[k].ap())
            for i in range(1, len(xin)):
                nc.sync.dma_start(out=xin[i].ap()[96:97, :],
                                  in_=cd["ones"].ap())
            if BPC > 1:
                load_image(1)

            for s in range(NI + OFF["outcopy"] + 1):
                # ---- PE ----
                if s < NI:
                    b, t = items[s]
                    xv = xin[b % 3].ap()
                    base = t * W
                    if fused_tr:
                        T_ps = psTR[s % 2].ap()[:, 0:384]
                    else:
                        T_ps = psT.tile([128, 384], F32)
                    for j in range(4):
                        nc.tensor.matmul(
                            T_ps[:, 96 * j:96 * j + 96],
                            xv[0:97, base + 128 * j:base + 128 * j + 128],
                            a1, start=True, stop=True)
                    st[s] = {"T_ps": T_ps, "b": b, "t": t}
                if 0 <= s - OFF["mm2"] < NI:
                    e = st[s - OFF["mm2"]]
                    D_ps = psD.tile([128, 384], F32)
                    for c in range(CCH):
                        for u in range(XX):
                            cu = c * 8 + u
                            nc.tensor.matmul(
                                D_ps[:, cu * 16:(cu + 1) * 16],
                                bdq[:, cu * 128:(cu + 1) * 128],
                                _t_cols(e["t_sb"], c, u),
                                start=True, stop=True)
                    e["D_ps"] = D_ps
                if 0 <= s - OFF["mm3"] < NI:
                    e = st[s - OFF["mm3"]]
                    if fused_tr:
                        R_ps = psTR[s % 2].ap()[0:P96, 384:384 + W]
                    else:
                        R_ps = psR.tile([P96, W], F32)
                    for j in range(4):
                        nc.tensor.matmul(
                            R_ps[:, 128 * j:128 * j + 128],
                            e["uu"][:, 96 * j:96 * j + 96],
                            bd, start=True, stop=True)
                    e["R_ps"] = R_ps
                if 0 <= s - OFF["mm4"] < NI:
                    e = st[s - OFF["mm4"]]
                    b, t = e["b"], e["t"]
                    xv = xin[b % 3].ap()
                    Y_ps = psY.tile([P96, W], F32)
                    if pe_add:
                        nc.tensor.matmul(Y_ps[:, :], ai2, e["rv"][0:P96, :],
                                         start=True, stop=False)
                        nc.tensor.matmul(Y_ps[:, :], i96,
                                         xv[0:P96, t * W:(t + 1) * W],
                                         start=False, stop=True)
                    else:
                        nc.tensor.matmul(Y_ps[:, :], ai2, e["rv"][0:P96, :],
                                         start=True, stop=True)
                    e["Y_ps"] = Y_ps

                def eng_copy(eng, dst, src_ap):
                    if eng == "vector":
                        nc.vector.tensor_copy(dst, src_ap)
                    else:
                        nc.scalar.copy(dst, src_ap)

                if fused_tr:
                    has_t = 0 <= s - OFF["tcopy"] < NI
                    has_r = 0 <= s - OFF["rcopy"] < NI
                    if has_t or has_r:
                        trv = work.tile([128, 896], F16, tag="trv")
                        p = (s - 1) % 2
                        if has_t and has_r:
                            nc.vector.tensor_copy(trv, psTR[p].ap()[:, 0:896])
                        elif has_t:
                            nc.vector.tensor_copy(
                                trv[:, 0:384], psTR[p].ap()[:, 0:384])
                        else:
                            nc.vector.tensor_copy(
                                trv[:, 384:896], psTR[p].ap()[:, 384:896])
                        if has_t:
                            st[s - OFF["tcopy"]]["t_sb"] = trv[:, 0:384]
                        if has_r:
                            st[s - OFF["rcopy"]]["rv"] = trv[0:P96, 384:896]
                else:
                    if 0 <= s - OFF["tcopy"] < NI:
                        e = st[s - OFF["tcopy"]]
                        t_sb = work.tile([128, 384], F16, tag="t_sb")
                        eng_copy(tcopy, t_sb, e["T_ps"][:, :])
                        e["t_sb"] = t_sb
                    if 0 <= s - OFF["rcopy"] < NI:
                        e = st[s - OFF["rcopy"]]
                        rv = work.tile([P96, W], F32R, tag="rv")
                        eng_copy(rcopy, rv, e["R_ps"][:, :])
                        e["rv"] = rv

                if 0 <= s - OFF["tanh"] < NI:
                    e = st[s - OFF["tanh"]]
                    tt = work.tile([128, 384], F16, tag="tt")
                    nc.scalar.activation(tt, e["D_ps"][:, :], ACTF.Tanh,
                                         bias=zb, scale=1.0)
                    e["tt"] = tt
                if 0 <= s - OFF["outcopy"] < NI:
                    e = st[s - OFF["outcopy"]]
                    ov = rout[e["b"] % 2].ap()
                    dst = ov[:, e["t"] * W:(e["t"] + 1) * W]
                    if pe_add:
                        eng_copy(outcopy, dst, e["Y_ps"][:, :])
                    else:
                        xv2 = xin[e["b"] % 3].ap()
                        nc.vector.tensor_tensor(
                            dst, e["Y_ps"][:, :],
                            xv2[0:P96, e["t"] * W:(e["t"] + 1) * W], AOT.add)
                    if e["t"] % 4 == 3:
                        store_chunk(e["b"], e["t"] - 3, 4)
                    del st[s - OFF["outcopy"]]

                if 0 <= s - OFF["uu"] < NI:
                    e = st[s - OFF["uu"]]
                    uu = work.tile([128, 384], F16, tag="uu")
                    eng = nc.gpsimd if uu_eng == "gpsimd" else nc.vector
                    ttv = e["tt"].rearrange("p (c u j g) -> p c u j g",
                                            c=3, u=8, j=4)
                    hqv = hq.rearrange("p (c u j g) -> p c u j g", c=3, u=8, j=4)
                    uuv = uu.rearrange("p (j c g u) -> p c u j g", j=4, c=3, g=4)
                    for c in range(CCH):
                        eng.tensor_tensor(uuv[:, c], ttv[:, c], hqv[:, c],
                                          AOT.mult)
                    e["uu"] = uu

                # prefetch image b+2 only once image b's LAST identity-add
                # (the final xin[b] reader) has been emitted -- the tile
                # framework orders the buffer overwrite after already-emitted
                # reads only.
                if 0 <= s - OFF["mm4"] < NI:
                    eb, et = items[s - OFF["mm4"]]
                    if et == NT - 1 and eb + 2 < n_images:
                        load_image(eb + 2)
    nc.compile()
    return nc


def _t_cols(t_sb, c, u):
    """Moving operand for MM2_cu: t columns {(j, c, g, u) : j, g} as
    [128, (j:4 stride 96), (g:4 stride 8)] at offset c*32+u."""
    return t_sb.rearrange("p (j cc g u) -> p j cc g u", j=4, cc=3, g=4)[
        :, :, c, :, u]


def _get_nc(**kw):
    key = tuple(sorted(kw.items()))
    if key not in _CACHE:
        _CACHE[key] = _build(**kw)
    return _CACHE[key]


def kernel(x, trace=False, **kw):
    from concourse import bass_utils
    nc = _get_nc(**kw)
    consts = _np_consts()
    up = {}
    for k, v in consts.items():
        if k == "ai2" and kw.get("fused_tr") is False:
            up[k] = v  # f32r = f32 bits
        else:
            up[k] = v.astype(np.float16)
    x = np.asarray(x)
    xh = x.astype(np.float16)
    in_maps = []
    for i in range(NCORES):
        m = {"x": xh[i * BPC:(i + 1) * BPC]}
        m.update(up)
        in_maps.append(m)
    res = bass_utils.run_bass_kernel_spmd(
        nc, in_maps, core_ids=list(range(NCORES)), trace=trace)
    _CACHE["last"] = res
    outs = np.concatenate([r["out"] for r in res.results], axis=0)
    return np.clip(outs.astype(np.float32), 0.0, 1.0)


def last_exec_time_ns():
    res = _CACHE.get("last")
    return None if res is None else res.exec_time_ns
